# revision 19
# baseline (speedup 1.0000x reference)
"""EncDec ConvLSTM kernel for 8 Trainium2 NeuronCores.

Sharding: 8 cores = 4 (batch) x 2 (spatial row-halves). Each core computes
its 32 output rows plus a shrinking redundant halo (21-s extra rows at
recurrent step s), so no cross-core communication is needed. Row-half 1
cores receive a vertically flipped image and ky-flipped conv weights, so a
single SPMD program serves all cores.

Conv3x3 is mapped to PE matmuls over pixels (N=512 free dim, bf16).
State tile R[128, grid]: partitions 0:64 hold h, partitions 64:128 hold h
col-shifted by +2 (one SBUF->SBUF DMA per tile, off the critical path).
Per 8-row tile and M-tile: 1 x-im2col MM (K=72), 3 paired-kx MMs (K=128,
reading [h | h+2col] at row offsets 0/1/2), and 3 middle-column taps as
K=64 row-strip MMs reading the same tile: ky=0,2 from the lower half at
col offset +1, ky=1 from the upper half at col offset -1 (the col-shifted
copy re-read one col left IS the middle column). No ta/tb packed copies.
The sig(i)*tanh(g) product is written cross-partition (in@64:128 ->
out@0:63) directly by the DVE, eliminating the old t1l DMA.
"""

import os
import sys

import numpy as np

for _p in ("/opt/trn_rl_repo", "/root/.axon_site/_ro/trn_rl_repo"):
    if os.path.isdir(_p) and _p not in sys.path:
        sys.path.append(_p)

T = 10
F = 8
HD = 64
HS = 64
WS = 64
NCORES = 8
PW = 66   # padded grid width/height
LEAD = 66  # one extra leading pad row in the R state tile
RSZ = LEAD + PW * PW + 2  # flat elems per partition in R
NSTEPS = 2 * T

_CACHE = {}


def _regions():
    """Rounded compute-region row counts per recurrent step s=1..NSTEPS."""
    out = []
    for s in range(1, NSTEPS + 1):
        need = NSTEPS + 1 - s
        rows = min(HS, 32 + need)
        rows = min(HS, ((rows + 7) // 8) * 8)
        out.append(rows)
    return out


def _build_program(use_bf16=True):
    from concourse import bacc, mybir, tile

    F32 = mybir.dt.float32
    MMDT = mybir.dt.bfloat16 if use_bf16 else mybir.dt.float32r
    ACT = mybir.ActivationFunctionType

    nc = bacc.Bacc("TRN2", target_bir_lowering=False, debug=False,
                   num_devices=NCORES)

    def din(name, shape, dt=MMDT):
        return nc.dram_tensor(name, shape, dt, kind="ExternalInput").ap()

    xe_d = din("xe", [T, F, PW, PW])
    xd_d = din("xd", [T, F, PW, PW])
    w_x = {"e": din("w_ex", [72, 256]), "d": din("w_dx", [72, 256])}
    w_p = {ph: [din(f"w_{ph}p{k}", [128, 256]) for k in range(3)]
           for ph in ("e", "d")}
    # middle-column (kx=1) taps: mA = [ky0 (strip 0); ky1 (strip 64)],
    # mB = [ky2] (strip 0, K=64); m1lo = ky1 at base 0 (non-strip variant)
    w_ma = {ph: din(f"w_{ph}ma", [128, 256]) for ph in ("e", "d")}
    w_mb = {ph: din(f"w_{ph}mb", [64, 256]) for ph in ("e", "d")}
    w_m1lo = {ph: din(f"w_{ph}m1lo", [64, 256]) for ph in ("e", "d")}
    w_op = [din(f"w_op{k}", [128, 8]) for k in range(3)]
    w_oma = din("w_oma", [128, 8])
    w_omb = din("w_omb", [64, 8])
    w_om1lo = din("w_om1lo", [64, 8])
    use_strip = os.environ.get("KERNEL_STRIP", "0") == "1"
    use_xbase = os.environ.get("KERNEL_XBASE", "1") == "1"
    b_m0 = {"e": din("b_e0", [128, 1], F32), "d": din("b_d0", [128, 1], F32)}
    b_m1 = {"e": din("b_e1", [128, 1], F32), "d": din("b_d1", [128, 1], F32)}
    b_o = din("b_o", [8, 1], F32)
    y_d = nc.dram_tensor("y", [T, F, 32, WS], F32, kind="ExternalOutput").ap()

    regions = _regions()

    with tile.TileContext(nc) as tc:
        with tc.tile_pool(name="wpool", bufs=1) as wp, \
             tc.tile_pool(name="state", bufs=1) as stp, \
             tc.tile_pool(name="x2p", bufs=2) as x2p, \
             tc.tile_pool(name="gps", bufs=3, space="PSUM") as gps, \
             tc.tile_pool(name="ops", bufs=1, space="PSUM") as ops, \
             tc.tile_pool(name="fip", bufs=3) as fip, \
             tc.tile_pool(name="ogp", bufs=3) as ogp, \
             tc.tile_pool(name="t1p", bufs=3) as t1p, \
             tc.tile_pool(name="thp", bufs=3) as thp, \
             tc.tile_pool(name="yyp", bufs=2) as yyp:

            # ---- warm-up weights from memset (no DMA dependency) ----
            wz = wp.tile([8, 8], MMDT, tag="wz")
            wz2 = wp.tile([8, 512], MMDT, tag="wz2")
            nc.vector.memset(wz[:], 0.0)
            nc.vector.memset(wz2[:], 0.0)
            # PE clock warm-up: ~3.4us of sustained matmul activity raises
            # the HAM clock gate to 8/8; these depend only on the two tiny
            # memsets, so they start ~1us in and cover the weight/x loads.
            for _ in range(32):
                wu = ops.tile([8, 1024], F32, tag="pso")
                nc.tensor.matmul(wu[:, 0:512], wz[:], wz2[:],
                                 start=True, stop=True)

            # ---- load weights / biases into SBUF ----
            # Issue order = sync-queue drain order: encoder first, then
            # out-conv, decoder last (not needed until step T+1).
            def wtile(src, shape, tag, dt=MMDT):
                t_ = wp.tile(shape, dt, tag=tag)
                nc.sync.dma_start(t_[:], src[:])
                return t_

            sw_x, sw_p, sw_ma, sw_mb, sw_m1lo, sb_m0, sb_m1 = \
                {}, {}, {}, {}, {}, {}, {}
            for ph in ("e", "d"):
                sw_x[ph] = wtile(w_x[ph], [72, 256], f"wx{ph}")
                sw_p[ph] = [wtile(w_p[ph][k], [128, 256], f"wp{ph}{k}")
                            for k in range(3)]
                sw_ma[ph] = wtile(w_ma[ph], [128, 256], f"wma{ph}")
                sw_mb[ph] = wtile(w_mb[ph], [64, 256], f"wmb{ph}")
                sw_m1lo[ph] = wtile(w_m1lo[ph], [64, 256], f"wm1lo{ph}")
                sb_m0[ph] = wtile(b_m0[ph], [128, 1], f"b0{ph}", F32)
                sb_m1[ph] = wtile(b_m1[ph], [128, 1], f"b1{ph}", F32)
                if ph == "e":
                    sw_op = [wtile(w_op[k], [128, 8], f"wop{k}")
                             for k in range(3)]
                    sw_oma = wtile(w_oma, [128, 8], "woma")
                    sw_omb = wtile(w_omb, [64, 8], "womb")
                    sw_om1lo = wtile(w_om1lo, [64, 8], "wom1lo")
                    sb_o = wtile(b_o, [8, 1], "bo", F32)

            # ---- persistent state ----
            # R: [h (parts 0:64) | h col-shifted +2 (parts 64:128)]
            rrA = stp.tile([128, RSZ], MMDT, tag="rrA")
            rrB = stp.tile([128, RSZ], MMDT, tag="rrB")
            c_t = stp.tile([64, HS * WS], F32, tag="c")
            nc.vector.memset(rrA[:], 0.0)
            nc.vector.memset(rrB[:], 0.0)
            nc.vector.memset(c_t[:], 0.0)

            def gview(t_, p0, p1, flat_off, nr=8):
                """[p1-p0, nr, 64] view of grid tile at flat elem offset."""
                v = t_[p0:p1, flat_off:flat_off + nr * PW]
                v = v.rearrange("p (r c) -> p r c", c=PW)
                return v[:, 0:nr, 0:64]

            def emit_x2col(s):
                """Load x im2col for step s: partition (ky*3+kx)*8+ic holds
                the flat padded image shifted by ky*66+kx (contiguous)."""
                ph = "e" if s <= T else "d"
                t_idx = (s - 1) if ph == "e" else (s - 1 - T)
                x_src = xe_d if ph == "e" else xd_d
                rp = regions[s - 1]
                ln = (rp - 1) * PW + 64
                x2 = x2p.tile([72, 57 * PW], MMDT, tag="x2")
                flat = x_src[t_idx].rearrange("a r c -> a (r c)")
                for tap in range(9):
                    sh = (tap // 3) * PW + (tap % 3)
                    nc.gpsimd.dma_start(x2[tap * 8:(tap + 1) * 8, 0:ln],
                                        flat[:, sh:sh + ln])
                return x2

            def gate_mms(ps, wx, wp3, wma, wmb, wm1lo, ms, x2v, R, r0,
                         skip_h):
                """Accumulate all 4H-gate conv taps for one M-tile."""
                nc.tensor.matmul(ps, wx[:, ms],
                                 x2v[0:72, r0:r0 + 8, 0:64],
                                 start=True, stop=skip_h)
                if skip_h:
                    return
                for k in range(3):
                    nc.tensor.matmul(
                        ps, wp3[k][:, ms],
                        gview(R, 0, 128, LEAD + (r0 + k) * PW),
                        start=False, stop=False)
                # middle column (kx=1) taps, all K=64 reads of the h half
                nc.tensor.matmul(ps, wma[0:64, ms],
                                 gview(R, 0, 64, LEAD + r0 * PW + 1),
                                 start=False, stop=False)
                if use_strip:
                    nc.tensor.matmul(
                        ps, wma[64:128, ms],
                        gview(R, 64, 128, LEAD + (r0 + 1) * PW - 1),
                        start=False, stop=False)
                else:
                    nc.tensor.matmul(
                        ps, wm1lo[:, ms],
                        gview(R, 0, 64, LEAD + (r0 + 1) * PW + 1),
                        start=False, stop=False)
                nc.tensor.matmul(ps, wmb[:, ms],
                                 gview(R, 0, 64, LEAD + (r0 + 2) * PW + 1),
                                 start=False, stop=True)

            def emit_outconv_pair(s, R, g):
                """relu(out conv) for decoder step s, rows 16g..16g+15."""
                t_o = s - 1 - T
                pso = ops.tile([8, 1024], F32, tag="pso")
                for hf in range(2):
                    r0 = g * 16 + hf * 8
                    d = pso[:, hf * 512:hf * 512 + 512]
                    for k in range(3):
                        nc.tensor.matmul(d, sw_op[k][:],
                                         gview(R, 0, 128, LEAD + (r0 + k) * PW),
                                         start=(k == 0), stop=False)
                    nc.tensor.matmul(d, sw_oma[0:64, :],
                                     gview(R, 0, 64, LEAD + r0 * PW + 1),
                                     start=False, stop=False)
                    if use_strip:
                        nc.tensor.matmul(d, sw_oma[64:128, :],
                                         gview(R, 64, 128,
                                               LEAD + (r0 + 1) * PW - 1),
                                         start=False, stop=False)
                    else:
                        nc.tensor.matmul(d, sw_om1lo[:, :],
                                         gview(R, 0, 64,
                                               LEAD + (r0 + 1) * PW + 1),
                                         start=False, stop=False)
                    nc.tensor.matmul(d, sw_omb[:, :],
                                     gview(R, 0, 64, LEAD + (r0 + 2) * PW + 1),
                                     start=False, stop=True)
                yy = yyp.tile([8, 1024], F32, tag="yy")
                nc.scalar.activation(yy[:], pso[:], ACT.Relu, bias=sb_o[:])
                nc.gpsimd.dma_start(
                    y_d[t_o, :, g * 16:g * 16 + 16, :],
                    yy[:].rearrange("p (r c) -> p r c", c=64))

            def gate_block(s, ph, R_r, R_w, x2v, r0, nt):
                """One epilogue block covering nt (1 or 2) 8-row tiles."""
                W = nt * 512
                skip_h = (s == 1)
                ps0 = gps.tile([128, 1024], F32, tag="ps")
                ps1 = gps.tile([128, 1024], F32, tag="ps")
                for i in range(nt):
                    sl = slice(i * 512, i * 512 + 512)
                    gate_mms(ps0[:, sl], sw_x[ph], sw_p[ph], sw_ma[ph],
                             sw_mb[ph], sw_m1lo[ph], slice(0, 128), x2v,
                             R_r, r0 + i * 8, skip_h)
                    gate_mms(ps1[:, sl], sw_x[ph], sw_p[ph], sw_ma[ph],
                             sw_mb[ph], sw_m1lo[ph], slice(128, 256), x2v,
                             R_r, r0 + i * 8, skip_h)

                # epilogue: M0=[f;i] M1=[o;g]
                fi = fip.tile([128, 1024], F32, tag="fi")
                og = ogp.tile([128, 1024], F32, tag="og")
                nc.scalar.activation(fi[:, 0:W], ps0[:, 0:W], ACT.Sigmoid,
                                     bias=sb_m0[ph][:])
                nc.scalar.activation(og[0:64, 0:W], ps1[0:64, 0:W],
                                     ACT.Sigmoid, bias=sb_m1[ph][0:64])
                nc.scalar.activation(og[64:128, 0:W], ps1[64:128, 0:W],
                                     ACT.Tanh, bias=sb_m1[ph][64:128])
                cs = c_t[:, r0 * 64:r0 * 64 + W]
                if skip_h and use_xbase:
                    # c==0 -> c = sigmoid(i)*tanh(g) directly (cross-base)
                    nc.vector.tensor_mul(cs, fi[64:128, 0:W],
                                         og[64:128, 0:W])
                else:
                    # t1 = sigmoid(i)*tanh(g): cross-partition DVE write
                    # (reads from parts 64:128, writes parts 0:64)
                    if use_xbase:
                        t1 = t1p.tile([64, 1024], F32, tag="t1")
                        nc.vector.tensor_mul(t1[:, 0:W], fi[64:128, 0:W],
                                             og[64:128, 0:W])
                    else:
                        t1u = t1p.tile([128, 1024], F32, tag="t1u")
                        nc.vector.tensor_mul(t1u[64:128, 0:W],
                                             fi[64:128, 0:W],
                                             og[64:128, 0:W])
                        t1 = t1p.tile([64, 1024], F32, tag="t1")
                        nc.sync.dma_start(t1[:, 0:W], t1u[64:128, 0:W])
                    nc.vector.tensor_mul(cs, cs, fi[0:64, 0:W])
                    nc.vector.tensor_add(cs, cs, t1[:, 0:W])
                # tail (tanh(c), h-mul, col-shift) is emitted one block
                # later by gate_tail so the ACT queue never head-of-line
                # blocks on the DVE c-update chain
                return (R_w, r0, W, og)

            def gate_tail(st):
                R_w, r0, W, og = st
                cs = c_t[:, r0 * 64:r0 * 64 + W]
                th = thp.tile([64, 1024], MMDT, tag="th")
                nc.scalar.activation(th[:, 0:W], cs, ACT.Tanh)
                # h = tanh(c) * sigmoid(o) -> lower half of write buffer
                thv = th[:, 0:W].rearrange("p (r c) -> p r c", c=64)
                nt8 = W // 64
                nc.vector.tensor_mul(
                    gview(R_w, 0, 64, LEAD + (r0 + 1) * PW + 1, nt8), thv,
                    og[0:64, 0:W].rearrange("p (r c) -> p r c", c=64))
                # col-shifted copy (+2) into partitions 64:128
                nc.sync.dma_start(
                    gview(R_w, 64, 128, LEAD + (r0 + 1) * PW - 1, nt8),
                    gview(R_w, 0, 64, LEAD + (r0 + 1) * PW + 1, nt8))

            pend_tail = None
            x2_cur = emit_x2col(1)
            for s in range(1, NSTEPS + 1):
                ph = "e" if s <= T else "d"
                rp = regions[s - 1]
                ntiles = rp // 8
                if s % 2 == 0:  # read buffers written at s-1
                    R_r, R_w = rrA, rrB
                else:
                    R_r, R_w = rrB, rrA

                x2v = x2_cur[:].rearrange("p (r c) -> p r c", c=PW)
                if s < NSTEPS:
                    x2_next = emit_x2col(s + 1)  # prefetch on gpsimd queue

                blocks = [(16 * i, 2) for i in range(ntiles // 2)]
                if ntiles % 2:
                    blocks.append((16 * (ntiles // 2), 1))
                for bi, (r0, nt) in enumerate(blocks):
                    st = gate_block(s, ph, R_r, R_w, x2v, r0, nt)
                    if pend_tail is not None:
                        gate_tail(pend_tail)
                    pend_tail = st
                    # prev decoder step's out conv, interleaved between
                    # gate blocks so its PSUM/ACT deps never stall the PE
                    if s > T + 1 and bi < 2:
                        emit_outconv_pair(s - 1, R_r, bi)

                if s < NSTEPS:
                    x2_cur = x2_next

            # out conv for the final decoder step (NSTEPS even -> B buffer)
            if pend_tail is not None:
                gate_tail(pend_tail)
            emit_outconv_pair(NSTEPS, rrB, 0)
            emit_outconv_pair(NSTEPS, rrB, 1)

    nc.compile()
    return nc


def _prep_core_inputs(core, enc_in, dec_in, enc_W, enc_b, dec_W, dec_b,
                      out_W, out_b, use_bf16=True):
    import ml_dtypes
    mm_np = ml_dtypes.bfloat16 if use_bf16 else np.float32
    b, half = core // 2, core % 2
    # gate permutation: [f, i, o, g]
    perm = np.concatenate([np.arange(0, 128), np.arange(192, 256),
                           np.arange(128, 192)])

    def prep_x(x):
        x = x[b]  # [T, F, 64, 64]
        if half:
            x = x[:, :, ::-1, :]
        xp = np.zeros((T, F, PW, PW), np.float32)
        xp[:, :, 1:65, 1:65] = x
        return np.ascontiguousarray(xp)

    def prep_gateW(W, bias):
        Wf = W[:, :, ::-1, :] if half else W
        Wp = np.ascontiguousarray(Wf[perm])  # [256, 72, 3, 3]
        bp = bias[perm].astype(np.float32)
        # x part: rows (ky*3+kx)*8+ic
        lx = Wp[:, :F].transpose(2, 3, 1, 0).reshape(72, 256)
        lp = [np.concatenate([Wp[:, F:, k, 0].T, Wp[:, F:, k, 2].T], axis=0)
              for k in range(3)]  # [128, 256]
        lma = np.concatenate([Wp[:, F:, 0, 1].T, Wp[:, F:, 1, 1].T],
                             axis=0)  # [128, 256]
        lmb = np.ascontiguousarray(Wp[:, F:, 2, 1].T)  # [64, 256]
        lm1 = np.ascontiguousarray(Wp[:, F:, 1, 1].T)  # [64, 256]
        return (np.ascontiguousarray(lx),
                [np.ascontiguousarray(a) for a in lp],
                np.ascontiguousarray(lma), lmb, lm1,
                np.ascontiguousarray(bp[0:128].reshape(128, 1)),
                np.ascontiguousarray(bp[128:256].reshape(128, 1)))

    ex, ep, ema, emb, em1, eb0, eb1 = prep_gateW(enc_W, enc_b)
    dx, dp, dma_, dmb, dm1, db0, db1 = prep_gateW(dec_W, dec_b)
    oWf = out_W[:, :, ::-1, :] if half else out_W
    op = [np.ascontiguousarray(np.concatenate(
        [oWf[:, :, k, 0].T, oWf[:, :, k, 2].T], axis=0).astype(np.float32))
        for k in range(3)]
    oma = np.ascontiguousarray(np.concatenate(
        [oWf[:, :, 0, 1].T, oWf[:, :, 1, 1].T], axis=0))
    omb = np.ascontiguousarray(oWf[:, :, 2, 1].T)
    om1 = np.ascontiguousarray(oWf[:, :, 1, 1].T)

    m = {"xe": prep_x(enc_in), "xd": prep_x(dec_in),
         "w_ex": ex, "w_dx": dx,
         "w_ema": ema, "w_emb": emb, "w_dma": dma_, "w_dmb": dmb,
         "w_em1lo": em1, "w_dm1lo": dm1,
         "w_oma": oma, "w_omb": omb, "w_om1lo": om1,
         "b_e0": eb0, "b_e1": eb1, "b_d0": db0, "b_d1": db1,
         "b_o": np.ascontiguousarray(out_b.reshape(8, 1).astype(np.float32))}
    for k in range(3):
        m[f"w_ep{k}"] = ep[k]
        m[f"w_dp{k}"] = dp[k]
        m[f"w_op{k}"] = op[k]
    f32_keys = {"b_e0", "b_e1", "b_d0", "b_d1", "b_o"}
    return {k: np.ascontiguousarray(np.asarray(
        v, np.float32 if k in f32_keys else mm_np)) for k, v in m.items()}


def _install_trace_hook():
    """Shim antenv.axon_hooks for NTFF profiling (dev only)."""
    import contextlib
    import ctypes
    import types

    so = "/opt/axon/libaxon_pjrt.so"
    if "antenv.axon_hooks" in sys.modules or not os.path.exists(so):
        return
    lib = ctypes.CDLL(so)
    if not hasattr(lib, "axon_start_nrt_profile"):
        return
    lib.axon_start_nrt_profile.argtypes = [ctypes.POINTER(ctypes.c_int64),
                                           ctypes.c_size_t]
    lib.axon_start_nrt_profile.restype = ctypes.c_int64
    lib.axon_stop_nrt_profile.argtypes = [ctypes.c_char_p]
    lib.axon_stop_nrt_profile.restype = ctypes.c_int64

    def _mk():
        @contextlib.contextmanager
        def _hook(output_dir, device_ids):
            import jax
            jax.devices()
            if device_ids:
                ids = (ctypes.c_int64 * len(device_ids))(*device_ids)
                rc = lib.axon_start_nrt_profile(ids, len(device_ids))
            else:
                rc = lib.axon_start_nrt_profile(None, 0)
            if rc != 0:
                raise RuntimeError(f"axon_start_nrt_profile rc={rc}")
            try:
                yield
            finally:
                lib.axon_stop_nrt_profile(str(output_dir).encode())
        return _hook

    mod = types.ModuleType("antenv.axon_hooks")
    mod.get_axon_ntff_profile_hook = _mk
    sys.modules["antenv.axon_hooks"] = mod


def kernel(enc_in, dec_in, enc_W, enc_b, dec_W, dec_b, out_W, out_b):
    from concourse.bass_utils import run_bass_kernel_spmd

    trace = os.environ.get("KERNEL_TRACE", "") == "1"
    if trace:
        _install_trace_hook()

    use_bf16 = os.environ.get("KERNEL_DTYPE", "bf16") != "f32r"
    if "nc" not in _CACHE:
        _CACHE["nc"] = _build_program(use_bf16)
    nc = _CACHE["nc"]

    args = (np.asarray(enc_in, np.float32), np.asarray(dec_in, np.float32),
            np.asarray(enc_W, np.float32), np.asarray(enc_b, np.float32),
            np.asarray(dec_W, np.float32), np.asarray(dec_b, np.float32),
            np.asarray(out_W, np.float32), np.asarray(out_b, np.float32))
    in_maps = [_prep_core_inputs(c, *args, use_bf16=use_bf16)
               for c in range(NCORES)]

    res = run_bass_kernel_spmd(nc, in_maps, list(range(NCORES)), trace=trace)
    if trace:
        _CACHE["exec_time_ns"] = res.exec_time_ns

    B = enc_in.shape[0]
    out = np.empty((B, T, F, HS, WS), np.float32)
    for c in range(NCORES):
        b, half = c // 2, c % 2
        yc = res.results[c]["y"]  # [T, F, 32, 64]
        if half:
            out[b, :, :, 32:64, :] = yc[:, :, ::-1, :]
        else:
            out[b, :, :, 0:32, :] = yc
    return out


# revision 24
# speedup vs baseline: 1.0183x; 1.0183x over previous
"""EncDec ConvLSTM kernel for 8 Trainium2 NeuronCores.

Sharding: 8 cores = 4 (batch) x 2 (spatial row-halves). Each core computes
its 32 output rows plus a shrinking redundant halo (21-s extra rows at
recurrent step s), so no cross-core communication is needed. Row-half 1
cores receive a vertically flipped image and ky-flipped conv weights, so a
single SPMD program serves all cores.

Conv3x3 is mapped to PE matmuls over pixels (N=512 free dim, bf16).
State tile R[128, grid]: partitions 0:64 hold h, partitions 64:128 hold h
col-shifted by +2 (one SBUF->SBUF DMA per tile, off the critical path).
Per 8-row tile and M-tile: 1 x-im2col MM (K=72), 3 paired-kx MMs (K=128,
reading [h | h+2col] at row offsets 0/1/2), and 3 middle-column taps as
K=64 row-strip MMs reading the same tile: ky=0,2 from the lower half at
col offset +1, ky=1 from the upper half at col offset -1 (the col-shifted
copy re-read one col left IS the middle column). No ta/tb packed copies.
The sig(i)*tanh(g) product is written cross-partition (in@64:128 ->
out@0:63) directly by the DVE, eliminating the old t1l DMA.
"""

import os
import sys

import numpy as np

for _p in ("/opt/trn_rl_repo", "/root/.axon_site/_ro/trn_rl_repo"):
    if os.path.isdir(_p) and _p not in sys.path:
        sys.path.append(_p)

T = 10
F = 8
HD = 64
HS = 64
WS = 64
NCORES = 8
PW = 66   # padded grid width/height
LEAD = 66  # one extra leading pad row in the R state tile
RSZ = LEAD + PW * PW + 2  # flat elems per partition in R
NSTEPS = 2 * T

_CACHE = {}


def _regions():
    """Rounded compute-region row counts per recurrent step s=1..NSTEPS."""
    out = []
    for s in range(1, NSTEPS + 1):
        need = NSTEPS + 1 - s
        rows = min(HS, 32 + need)
        rows = min(HS, ((rows + 7) // 8) * 8)
        out.append(rows)
    return out


def _build_program(use_bf16=True):
    from concourse import bacc, mybir, tile

    F32 = mybir.dt.float32
    MMDT = mybir.dt.bfloat16 if use_bf16 else mybir.dt.float32r
    ACT = mybir.ActivationFunctionType

    nc = bacc.Bacc("TRN2", target_bir_lowering=False, debug=False,
                   num_devices=NCORES)

    def din(name, shape, dt=MMDT):
        return nc.dram_tensor(name, shape, dt, kind="ExternalInput").ap()

    xe_d = din("xe", [T, F, PW, PW])
    xd_d = din("xd", [T, F, PW, PW])
    w_x = {"e": din("w_ex", [72, 256]), "d": din("w_dx", [72, 256])}
    w_p = {ph: [din(f"w_{ph}p{k}", [128, 256]) for k in range(3)]
           for ph in ("e", "d")}
    # middle-column (kx=1) taps: mA = [ky0 (strip 0); ky1 (strip 64)],
    # mB = [ky2] (strip 0, K=64); m1lo = ky1 at base 0 (non-strip variant)
    w_ma = {ph: din(f"w_{ph}ma", [128, 256]) for ph in ("e", "d")}
    w_mb = {ph: din(f"w_{ph}mb", [64, 256]) for ph in ("e", "d")}
    w_m1lo = {ph: din(f"w_{ph}m1lo", [64, 256]) for ph in ("e", "d")}
    w_op = [din(f"w_op{k}", [128, 8]) for k in range(3)]
    w_oma = din("w_oma", [128, 8])
    w_omb = din("w_omb", [64, 8])
    w_om1lo = din("w_om1lo", [64, 8])
    use_strip = os.environ.get("KERNEL_STRIP", "0") == "1"
    use_xbase = os.environ.get("KERNEL_XBASE", "1") == "1"
    b_m0 = {"e": din("b_e0", [128, 1], F32), "d": din("b_d0", [128, 1], F32)}
    b_m1 = {"e": din("b_e1", [128, 1], F32), "d": din("b_d1", [128, 1], F32)}
    b_o = din("b_o", [8, 1], F32)
    y_d = nc.dram_tensor("y", [T, F, 32, WS], F32, kind="ExternalOutput").ap()

    regions = _regions()

    with tile.TileContext(nc) as tc:
        with tc.tile_pool(name="wpool", bufs=1) as wp, \
             tc.tile_pool(name="state", bufs=1) as stp, \
             tc.tile_pool(name="x2p", bufs=2) as x2p, \
             tc.tile_pool(name="gps", bufs=6, space="PSUM") as gps, \
             tc.tile_pool(name="ops", bufs=2, space="PSUM") as ops, \
             tc.tile_pool(name="fip", bufs=3) as fip, \
             tc.tile_pool(name="ogp", bufs=3) as ogp, \
             tc.tile_pool(name="t1p", bufs=3) as t1p, \
             tc.tile_pool(name="thp", bufs=3) as thp, \
             tc.tile_pool(name="yyp", bufs=2) as yyp:

            # ---- warm-up weights from memset (no DMA dependency) ----
            wz = wp.tile([8, 8], MMDT, tag="wz")
            wz2 = wp.tile([8, 512], MMDT, tag="wz2")
            nc.vector.memset(wz[:], 0.0)
            nc.vector.memset(wz2[:], 0.0)
            # PE clock warm-up: ~3.4us of sustained matmul activity raises
            # the HAM clock gate to 8/8; these depend only on the two tiny
            # memsets, so they start ~1us in and cover the weight/x loads.
            for _ in range(32):
                wu = ops.tile([8, 512], F32, tag="pso")
                nc.tensor.matmul(wu[:], wz[:], wz2[:],
                                 start=True, stop=True)

            # ---- load weights / biases into SBUF ----
            # Issue order = sync-queue drain order: encoder first, then
            # out-conv, decoder last (not needed until step T+1).
            def wtile(src, shape, tag, dt=MMDT):
                t_ = wp.tile(shape, dt, tag=tag)
                nc.sync.dma_start(t_[:], src[:])
                return t_

            sw_x, sw_p, sw_ma, sw_mb, sw_m1lo, sb_m0, sb_m1 = \
                {}, {}, {}, {}, {}, {}, {}
            for ph in ("e", "d"):
                sw_x[ph] = wtile(w_x[ph], [72, 256], f"wx{ph}")
                sw_p[ph] = [wtile(w_p[ph][k], [128, 256], f"wp{ph}{k}")
                            for k in range(3)]
                sw_ma[ph] = wtile(w_ma[ph], [128, 256], f"wma{ph}")
                sw_mb[ph] = wtile(w_mb[ph], [64, 256], f"wmb{ph}")
                sw_m1lo[ph] = wtile(w_m1lo[ph], [64, 256], f"wm1lo{ph}")
                sb_m0[ph] = wtile(b_m0[ph], [128, 1], f"b0{ph}", F32)
                sb_m1[ph] = wtile(b_m1[ph], [128, 1], f"b1{ph}", F32)
                if ph == "e":
                    sw_op = [wtile(w_op[k], [128, 8], f"wop{k}")
                             for k in range(3)]
                    sw_oma = wtile(w_oma, [128, 8], "woma")
                    sw_omb = wtile(w_omb, [64, 8], "womb")
                    sw_om1lo = wtile(w_om1lo, [64, 8], "wom1lo")
                    sb_o = wtile(b_o, [8, 1], "bo", F32)

            # ---- persistent state ----
            # R: [h (parts 0:64) | h col-shifted +2 (parts 64:128)]
            rrA = stp.tile([128, RSZ], MMDT, tag="rrA")
            rrB = stp.tile([128, RSZ], MMDT, tag="rrB")
            c_t = stp.tile([64, HS * WS], F32, tag="c")
            nc.vector.memset(rrA[:], 0.0)
            nc.vector.memset(rrB[:], 0.0)
            nc.vector.memset(c_t[:], 0.0)

            def gview(t_, p0, p1, flat_off, nr=8):
                """[p1-p0, nr, 64] view of grid tile at flat elem offset."""
                v = t_[p0:p1, flat_off:flat_off + nr * PW]
                v = v.rearrange("p (r c) -> p r c", c=PW)
                return v[:, 0:nr, 0:64]

            def emit_x2col(s):
                """Load x im2col for step s: partition (ky*3+kx)*8+ic holds
                the flat padded image shifted by ky*66+kx (contiguous)."""
                ph = "e" if s <= T else "d"
                t_idx = (s - 1) if ph == "e" else (s - 1 - T)
                x_src = xe_d if ph == "e" else xd_d
                rp = regions[s - 1]
                ln = (rp - 1) * PW + 64
                x2 = x2p.tile([72, 57 * PW], MMDT, tag="x2")
                flat = x_src[t_idx].rearrange("a r c -> a (r c)")
                for tap in range(9):
                    sh = (tap // 3) * PW + (tap % 3)
                    nc.gpsimd.dma_start(x2[tap * 8:(tap + 1) * 8, 0:ln],
                                        flat[:, sh:sh + ln])
                return x2

            def gate_mms(ps, wx, wp3, wma, wmb, wm1lo, ms, x2v, R, r0,
                         skip_h):
                """Accumulate all 4H-gate conv taps for one M-tile."""
                nc.tensor.matmul(ps, wx[:, ms],
                                 x2v[0:72, r0:r0 + 8, 0:64],
                                 start=True, stop=skip_h)
                if skip_h:
                    return
                for k in range(3):
                    nc.tensor.matmul(
                        ps, wp3[k][:, ms],
                        gview(R, 0, 128, LEAD + (r0 + k) * PW),
                        start=False, stop=False)
                # middle column (kx=1) taps, all K=64 reads of the h half
                nc.tensor.matmul(ps, wma[0:64, ms],
                                 gview(R, 0, 64, LEAD + r0 * PW + 1),
                                 start=False, stop=False)
                if use_strip:
                    nc.tensor.matmul(
                        ps, wma[64:128, ms],
                        gview(R, 64, 128, LEAD + (r0 + 1) * PW - 1),
                        start=False, stop=False)
                else:
                    nc.tensor.matmul(
                        ps, wm1lo[:, ms],
                        gview(R, 0, 64, LEAD + (r0 + 1) * PW + 1),
                        start=False, stop=False)
                nc.tensor.matmul(ps, wmb[:, ms],
                                 gview(R, 0, 64, LEAD + (r0 + 2) * PW + 1),
                                 start=False, stop=True)

            def emit_outconv1(s, R, n2):
                """relu(out conv) for decoder step s, rows 8*n2..8*n2+7."""
                t_o = s - 1 - T
                r0 = n2 * 8
                pso = ops.tile([8, 512], F32, tag="pso")
                for k in range(3):
                    nc.tensor.matmul(pso[:], sw_op[k][:],
                                     gview(R, 0, 128, LEAD + (r0 + k) * PW),
                                     start=(k == 0), stop=False)
                nc.tensor.matmul(pso[:], sw_oma[0:64, :],
                                 gview(R, 0, 64, LEAD + r0 * PW + 1),
                                 start=False, stop=False)
                if use_strip:
                    nc.tensor.matmul(pso[:], sw_oma[64:128, :],
                                     gview(R, 64, 128,
                                           LEAD + (r0 + 1) * PW - 1),
                                     start=False, stop=False)
                else:
                    nc.tensor.matmul(pso[:], sw_om1lo[:, :],
                                     gview(R, 0, 64,
                                           LEAD + (r0 + 1) * PW + 1),
                                     start=False, stop=False)
                nc.tensor.matmul(pso[:], sw_omb[:, :],
                                 gview(R, 0, 64, LEAD + (r0 + 2) * PW + 1),
                                 start=False, stop=True)
                yy = yyp.tile([8, 512], F32, tag="yy")
                nc.scalar.activation(yy[:], pso[:], ACT.Relu, bias=sb_o[:])
                nc.gpsimd.dma_start(
                    y_d[t_o, :, r0:r0 + 8, :],
                    yy[:].rearrange("p (r c) -> p r c", c=64))

            def gate_block(s, ph, R_r, R_w, x2v, r0):
                """Gate conv + c-update for one 8-row tile."""
                skip_h = False
                ps0 = gps.tile([128, 512], F32, tag="ps")
                ps1 = gps.tile([128, 512], F32, tag="ps")
                gate_mms(ps0[:], sw_x[ph], sw_p[ph], sw_ma[ph],
                         sw_mb[ph], sw_m1lo[ph], slice(0, 128), x2v,
                         R_r, r0, skip_h)
                gate_mms(ps1[:], sw_x[ph], sw_p[ph], sw_ma[ph],
                         sw_mb[ph], sw_m1lo[ph], slice(128, 256), x2v,
                         R_r, r0, skip_h)

                # epilogue: M0=[f;i] M1=[o;g]
                fi = fip.tile([128, 512], F32, tag="fi")
                og = ogp.tile([128, 512], F32, tag="og")
                nc.scalar.activation(fi[:], ps0[:], ACT.Sigmoid,
                                     bias=sb_m0[ph][:])
                nc.scalar.activation(og[0:64], ps1[0:64],
                                     ACT.Sigmoid, bias=sb_m1[ph][0:64])
                nc.scalar.activation(og[64:128], ps1[64:128],
                                     ACT.Tanh, bias=sb_m1[ph][64:128])
                cs = c_t[:, r0 * 64:r0 * 64 + 512]
                # t1 = sigmoid(i)*tanh(g): cross-partition DVE write
                # (reads from parts 64:128, writes parts 0:64)
                if use_xbase:
                    t1 = t1p.tile([64, 512], F32, tag="t1")
                    nc.vector.tensor_mul(t1[:], fi[64:128], og[64:128])
                else:
                    t1u = t1p.tile([128, 512], F32, tag="t1u")
                    nc.vector.tensor_mul(t1u[64:128], fi[64:128],
                                         og[64:128])
                    t1 = t1p.tile([64, 512], F32, tag="t1")
                    nc.sync.dma_start(t1[:], t1u[64:128])
                nc.vector.tensor_mul(cs, cs, fi[0:64])
                nc.vector.tensor_add(cs, cs, t1[:])
                # tail (tanh(c), h-mul, col-shift) is emitted one tile
                # later by gate_tail so the ACT queue never head-of-line
                # blocks on the DVE c-update chain
                return (R_w, r0, og)

            def gate_tail(st):
                R_w, r0, og = st
                cs = c_t[:, r0 * 64:r0 * 64 + 512]
                th = thp.tile([64, 512], MMDT, tag="th")
                nc.scalar.activation(th[:], cs, ACT.Tanh)
                # h = tanh(c) * sigmoid(o) -> lower half of write buffer
                thv = th[:].rearrange("p (r c) -> p r c", c=64)
                nc.vector.tensor_mul(
                    gview(R_w, 0, 64, LEAD + (r0 + 1) * PW + 1), thv,
                    og[0:64].rearrange("p (r c) -> p r c", c=64))
                # col-shifted copy (+2) into partitions 64:128
                nc.sync.dma_start(
                    gview(R_w, 64, 128, LEAD + (r0 + 1) * PW - 1),
                    gview(R_w, 0, 64, LEAD + (r0 + 1) * PW + 1))

            pend_tail = None
            x2_cur = emit_x2col(1)
            for s in range(1, NSTEPS + 1):
                ph = "e" if s <= T else "d"
                rp = regions[s - 1]
                ntiles = rp // 8
                if s % 2 == 0:  # read buffers written at s-1
                    R_r, R_w = rrA, rrB
                else:
                    R_r, R_w = rrB, rrA

                x2v = x2_cur[:].rearrange("p (r c) -> p r c", c=PW)
                if s < NSTEPS:
                    x2_next = emit_x2col(s + 1)  # prefetch on gpsimd queue

                for n in range(ntiles):
                    st = gate_block(s, ph, R_r, R_w, x2v, 8 * n)
                    if pend_tail is not None:
                        gate_tail(pend_tail)
                    pend_tail = st
                    # prev decoder step's out conv, interleaved between
                    # gate tiles so its PSUM/ACT deps never stall the PE
                    if s > T + 1 and n < 4:
                        emit_outconv1(s - 1, R_r, n)

                if s < NSTEPS:
                    x2_cur = x2_next

            # out conv for the final decoder step (NSTEPS even -> B buffer)
            if pend_tail is not None:
                gate_tail(pend_tail)
            for n2 in range(4):
                emit_outconv1(NSTEPS, rrB, n2)

    nc.compile()
    return nc


def _prep_core_inputs(core, enc_in, dec_in, enc_W, enc_b, dec_W, dec_b,
                      out_W, out_b, use_bf16=True):
    import ml_dtypes
    mm_np = ml_dtypes.bfloat16 if use_bf16 else np.float32
    b, half = core // 2, core % 2
    # gate permutation: [f, i, o, g]
    perm = np.concatenate([np.arange(0, 128), np.arange(192, 256),
                           np.arange(128, 192)])

    def prep_x(x):
        x = x[b]  # [T, F, 64, 64]
        if half:
            x = x[:, :, ::-1, :]
        xp = np.zeros((T, F, PW, PW), np.float32)
        xp[:, :, 1:65, 1:65] = x
        return np.ascontiguousarray(xp)

    def prep_gateW(W, bias):
        Wf = W[:, :, ::-1, :] if half else W
        Wp = np.ascontiguousarray(Wf[perm])  # [256, 72, 3, 3]
        bp = bias[perm].astype(np.float32)
        # x part: rows (ky*3+kx)*8+ic
        lx = Wp[:, :F].transpose(2, 3, 1, 0).reshape(72, 256)
        lp = [np.concatenate([Wp[:, F:, k, 0].T, Wp[:, F:, k, 2].T], axis=0)
              for k in range(3)]  # [128, 256]
        lma = np.concatenate([Wp[:, F:, 0, 1].T, Wp[:, F:, 1, 1].T],
                             axis=0)  # [128, 256]
        lmb = np.ascontiguousarray(Wp[:, F:, 2, 1].T)  # [64, 256]
        lm1 = np.ascontiguousarray(Wp[:, F:, 1, 1].T)  # [64, 256]
        return (np.ascontiguousarray(lx),
                [np.ascontiguousarray(a) for a in lp],
                np.ascontiguousarray(lma), lmb, lm1,
                np.ascontiguousarray(bp[0:128].reshape(128, 1)),
                np.ascontiguousarray(bp[128:256].reshape(128, 1)))

    ex, ep, ema, emb, em1, eb0, eb1 = prep_gateW(enc_W, enc_b)
    dx, dp, dma_, dmb, dm1, db0, db1 = prep_gateW(dec_W, dec_b)
    oWf = out_W[:, :, ::-1, :] if half else out_W
    op = [np.ascontiguousarray(np.concatenate(
        [oWf[:, :, k, 0].T, oWf[:, :, k, 2].T], axis=0).astype(np.float32))
        for k in range(3)]
    oma = np.ascontiguousarray(np.concatenate(
        [oWf[:, :, 0, 1].T, oWf[:, :, 1, 1].T], axis=0))
    omb = np.ascontiguousarray(oWf[:, :, 2, 1].T)
    om1 = np.ascontiguousarray(oWf[:, :, 1, 1].T)

    m = {"xe": prep_x(enc_in), "xd": prep_x(dec_in),
         "w_ex": ex, "w_dx": dx,
         "w_ema": ema, "w_emb": emb, "w_dma": dma_, "w_dmb": dmb,
         "w_em1lo": em1, "w_dm1lo": dm1,
         "w_oma": oma, "w_omb": omb, "w_om1lo": om1,
         "b_e0": eb0, "b_e1": eb1, "b_d0": db0, "b_d1": db1,
         "b_o": np.ascontiguousarray(out_b.reshape(8, 1).astype(np.float32))}
    for k in range(3):
        m[f"w_ep{k}"] = ep[k]
        m[f"w_dp{k}"] = dp[k]
        m[f"w_op{k}"] = op[k]
    f32_keys = {"b_e0", "b_e1", "b_d0", "b_d1", "b_o"}
    return {k: np.ascontiguousarray(np.asarray(
        v, np.float32 if k in f32_keys else mm_np)) for k, v in m.items()}


def _install_trace_hook():
    """Shim antenv.axon_hooks for NTFF profiling (dev only)."""
    import contextlib
    import ctypes
    import types

    so = "/opt/axon/libaxon_pjrt.so"
    if "antenv.axon_hooks" in sys.modules or not os.path.exists(so):
        return
    lib = ctypes.CDLL(so)
    if not hasattr(lib, "axon_start_nrt_profile"):
        return
    lib.axon_start_nrt_profile.argtypes = [ctypes.POINTER(ctypes.c_int64),
                                           ctypes.c_size_t]
    lib.axon_start_nrt_profile.restype = ctypes.c_int64
    lib.axon_stop_nrt_profile.argtypes = [ctypes.c_char_p]
    lib.axon_stop_nrt_profile.restype = ctypes.c_int64

    def _mk():
        @contextlib.contextmanager
        def _hook(output_dir, device_ids):
            import jax
            jax.devices()
            if device_ids:
                ids = (ctypes.c_int64 * len(device_ids))(*device_ids)
                rc = lib.axon_start_nrt_profile(ids, len(device_ids))
            else:
                rc = lib.axon_start_nrt_profile(None, 0)
            if rc != 0:
                raise RuntimeError(f"axon_start_nrt_profile rc={rc}")
            try:
                yield
            finally:
                lib.axon_stop_nrt_profile(str(output_dir).encode())
        return _hook

    mod = types.ModuleType("antenv.axon_hooks")
    mod.get_axon_ntff_profile_hook = _mk
    sys.modules["antenv.axon_hooks"] = mod


def kernel(enc_in, dec_in, enc_W, enc_b, dec_W, dec_b, out_W, out_b):
    from concourse.bass_utils import run_bass_kernel_spmd

    trace = os.environ.get("KERNEL_TRACE", "") == "1"
    if trace:
        _install_trace_hook()

    use_bf16 = os.environ.get("KERNEL_DTYPE", "bf16") != "f32r"
    if "nc" not in _CACHE:
        _CACHE["nc"] = _build_program(use_bf16)
    nc = _CACHE["nc"]

    args = (np.asarray(enc_in, np.float32), np.asarray(dec_in, np.float32),
            np.asarray(enc_W, np.float32), np.asarray(enc_b, np.float32),
            np.asarray(dec_W, np.float32), np.asarray(dec_b, np.float32),
            np.asarray(out_W, np.float32), np.asarray(out_b, np.float32))
    in_maps = [_prep_core_inputs(c, *args, use_bf16=use_bf16)
               for c in range(NCORES)]

    res = run_bass_kernel_spmd(nc, in_maps, list(range(NCORES)), trace=trace)
    if trace:
        _CACHE["exec_time_ns"] = res.exec_time_ns

    B = enc_in.shape[0]
    out = np.empty((B, T, F, HS, WS), np.float32)
    for c in range(NCORES):
        b, half = c // 2, c % 2
        yc = res.results[c]["y"]  # [T, F, 32, 64]
        if half:
            out[b, :, :, 32:64, :] = yc[:, :, ::-1, :]
        else:
            out[b, :, :, 0:32, :] = yc
    return out
